# revision 1
# baseline (speedup 1.0000x reference)
import numpy as np
import concourse.bacc as bacc
import concourse.mybir as mybir
from concourse.tile import TileContext
from concourse.bass_utils import run_bass_kernel_spmd

DIM_INPUT = 128
DIM_REC = 512
DIM_OUT = 256
BATCH = 512
NCORES = 8
B = BATCH // NCORES  # 64 per-core batch
T = DIM_INPUT        # 128 timesteps
KJ = DIM_REC // 128  # 4 chunks of the recurrent dim
OJ = DIM_OUT // 128  # 2 chunks of the output dim

F32 = mybir.dt.float32
MMDT = mybir.dt.float16  # matmul operand dtype (FWL + 1 cyc/row on PE)
MMNP = np.float16

# MM issue order within a step. 's{j}' is the x-projection matmul for
# group j (start=True seeds psum bank j); (j,k) accumulates Wh[k->j]@g_k.
# Order from discrete-event search over the epilogue dependency chain
# (sched_search.py, scalar handles groups 0,1 / vector 2,3); model
# period 915ns vs 1003ns for the previous hand schedule.
STEP_ORDER = ['s1', 's2', 's0', 's3', (3, 0), (2, 0), (0, 2), (0, 0),
              (1, 2), (2, 2), (0, 3), (0, 1), (2, 3), (2, 1), (1, 3),
              (1, 0), (1, 1), (3, 3), (3, 1), (3, 2)]


def _build_nc():
    nc = bacc.Bacc("TRN2", target_bir_lowering=False, debug=False,
                   num_devices=NCORES)
    xT = nc.dram_tensor("xT", [DIM_INPUT, B], MMDT, kind="ExternalInput")
    WhT = nc.dram_tensor("WhT", [DIM_REC, DIM_REC], MMDT, kind="ExternalInput")
    WxT = nc.dram_tensor("WxT", [DIM_INPUT, DIM_REC], MMDT, kind="ExternalInput")
    whyR = nc.dram_tensor("whyR", [128, KJ * DIM_OUT], MMDT, kind="ExternalInput")
    bcR = nc.dram_tensor("bcR", [128, KJ], F32, kind="ExternalInput")
    byR = nc.dram_tensor("byR", [128, OJ], F32, kind="ExternalInput")
    yT = nc.dram_tensor("yT", [DIM_OUT, B], F32, kind="ExternalOutput")

    RELU = mybir.ActivationFunctionType.Relu
    IDENT = mybir.ActivationFunctionType.Identity
    ADD = mybir.AluOpType.add
    MAX = mybir.AluOpType.max

    with TileContext(nc) as tc:
        with tc.tile_pool(name="w", bufs=1) as wp, \
             tc.tile_pool(name="s", bufs=1) as sp, \
             tc.psum_pool(name="p", bufs=1) as pp:
            wh = [wp.tile([128, DIM_REC], MMDT, name=f"wh{k}") for k in range(KJ)]
            wx = wp.tile([128, DIM_REC], MMDT, name="wx")
            whyt = wp.tile([128, KJ * DIM_OUT], MMDT, name="why")
            bct = wp.tile([128, KJ], F32, name="bct")
            byt = wp.tile([128, OJ], F32, name="byt")
            xt = sp.tile([128, B], MMDT, name="xt")
            g = [[sp.tile([128, B], MMDT, name=f"g{p}_{k}") for k in range(KJ)]
                 for p in range(2)]
            ps = [[pp.tile([128, B], F32, name=f"ps{p}_{j}") for j in range(KJ)]
                  for p in range(2)]
            psy = [ps[0][0], ps[0][1]]  # reuse phase-0 banks (free after step T-1)

            # startup DMAs: big weight loads split across the three
            # DMA-capable queues; tail-only tensors (why/byt) trail on sync.
            nc.sync.dma_start(out=xt[:], in_=xT[:])
            nc.sync.dma_start(out=wx[0:64, :], in_=WxT[0:64, :])
            nc.scalar.dma_start(out=wx[64:128, :], in_=WxT[64:128, :])
            nc.sync.dma_start(out=bct[:], in_=bcR[:])
            nc.gpsimd.dma_start(out=wh[0][:], in_=WhT[0:128, :])
            nc.scalar.dma_start(out=wh[1][:], in_=WhT[128:256, :])
            nc.gpsimd.dma_start(out=wh[2][:], in_=WhT[256:384, :])
            nc.scalar.dma_start(out=wh[3][:], in_=WhT[384:512, :])
            nc.sync.dma_start(out=whyt[:], in_=whyR[:])
            nc.sync.dma_start(out=byt[:], in_=byR[:])

            def epilogue(dst, psrc):
                # dst_j = relu(psum_j + bc_j); scalar takes 0,1 / vector 2,3
                nc.scalar.activation(dst[0][:], psrc[0][:], RELU,
                                     bias=bct[:, 0:1])
                nc.scalar.activation(dst[1][:], psrc[1][:], RELU,
                                     bias=bct[:, 1:2])
                nc.vector.tensor_scalar(dst[2][:], psrc[2][:],
                                        bct[:, 2:3], 0.0, ADD, MAX)
                nc.vector.tensor_scalar(dst[3][:], psrc[3][:],
                                        bct[:, 3:4], 0.0, ADD, MAX)

            # step 1 (h0 = 0): g0_j = relu((x @ W_x2h.T).T[j] + bc[j])
            for j in range(KJ):
                nc.tensor.matmul(ps[0][j][:], wx[:, j * 128:(j + 1) * 128],
                                 xt[:], start=True, stop=True)
            epilogue(g[0], ps[0])

            # 127 recurrent steps: g' = relu(x @ Wx + Wh @ g + bc)
            for s in range(1, T):
                cur, nxt = g[(s + 1) % 2], g[s % 2]
                pcur = ps[s % 2]
                grp = [0] * KJ
                for it in STEP_ORDER:
                    if isinstance(it, str):
                        j = int(it[1])
                        nc.tensor.matmul(pcur[j][:],
                                         wx[:, j * 128:(j + 1) * 128],
                                         xt[:], start=True, stop=False)
                    else:
                        j, k = it
                        grp[j] += 1
                        nc.tensor.matmul(pcur[j][:],
                                         wh[k][:, j * 128:(j + 1) * 128],
                                         cur[k][:], start=False,
                                         stop=(grp[j] == KJ))
                epilogue(nxt, pcur)

            gfin = g[(T - 1) % 2]
            # yT[jslice] = W_h2y[jslice] @ h.T + b_h2y[jslice]
            for j in range(OJ):
                for k in range(KJ):
                    nc.tensor.matmul(
                        psy[j][:],
                        whyt[:, k * DIM_OUT + j * 128:k * DIM_OUT + (j + 1) * 128],
                        gfin[k][:], start=(k == 0), stop=(k == KJ - 1))
            ytile = [sp.tile([128, B], F32, name=f"yt{j}") for j in range(OJ)]
            nc.scalar.activation(ytile[0][:], psy[0][:], IDENT,
                                 bias=byt[:, 0:1])
            nc.vector.tensor_scalar(ytile[1][:], psy[1][:], byt[:, 1:2],
                                    None, ADD)
            nc.sync.dma_start(out=yT[0:128, :], in_=ytile[0][:])
            nc.gpsimd.dma_start(out=yT[128:256, :], in_=ytile[1][:])

    nc.compile()
    return nc


_NC = None
TRACE = False
TRACE_TMPDIR = None
LAST_RESULTS = None


def kernel(x, W_x2h, b_x2h, W_h2h, b_h2h, W_h2y, b_h2y):
    global _NC, LAST_RESULTS
    if _NC is None:
        _NC = _build_nc()

    x = np.asarray(x, np.float32)
    WhyT = np.asarray(W_h2y, np.float32).T.astype(MMNP)
    bc = np.asarray(b_x2h, np.float32) + np.asarray(b_h2h, np.float32)
    shared = {
        "WhT": np.ascontiguousarray(np.asarray(W_h2h, np.float32).T.astype(MMNP)),
        "WxT": np.ascontiguousarray(np.asarray(W_x2h, np.float32).T.astype(MMNP)),
        "whyR": np.ascontiguousarray(np.concatenate(
            [WhyT[k * 128:(k + 1) * 128, :] for k in range(KJ)], axis=1)),
        "bcR": np.ascontiguousarray(bc.reshape(KJ, 128).T),
        "byR": np.ascontiguousarray(
            np.asarray(b_h2y, np.float32).reshape(OJ, 128).T),
    }
    ins = []
    for i in range(NCORES):
        m = dict(shared)
        m["xT"] = np.ascontiguousarray(x[i * B:(i + 1) * B, :].T.astype(MMNP))
        ins.append(m)

    kw = {}
    if TRACE:
        kw = {"trace": True, "tmpdir": TRACE_TMPDIR}
    res = run_bass_kernel_spmd(_NC, ins, core_ids=list(range(NCORES)), **kw)
    LAST_RESULTS = res
    out = np.empty((BATCH, DIM_OUT), np.float32)
    for i in range(NCORES):
        out[i * B:(i + 1) * B, :] = res.results[i]["yT"].T
    return out



# revision 4
# speedup vs baseline: 1.0294x; 1.0294x over previous
"""RNN kernel v2: identity-seeded PSUM + bias-free epilogue, config-driven.

  - Host precomputes xhb = x@Wx.T + b_x2h + b_h2h (f32) and h1 = relu(xhb);
    device runs steps 2..T and the output projection.
  - Each step's PSUM windows are seeded with xhb via identity matmuls
    (lhsT = xhbT chunk [64,128], rhs = I64), so the epilogue is a pure relu
    (no bias) and epilogue ops may span 2 windows ([128,128]).
  - HW rule: within one PSUM tile, accumulation groups (seed..stop) must be
    sequential; across tiles they interleave freely. PSUM tile structure
    derives from EPI_CFG.
  - EPI_CFG / TEMPLATE come from sim.py's schedule search.
"""
import numpy as np
import concourse.bacc as bacc
import concourse.mybir as mybir
from concourse.tile import TileContext
from concourse.bass_utils import run_bass_kernel_spmd

DIM_INPUT = 128
DIM_REC = 512
DIM_OUT = 256
BATCH = 512
NCORES = 8
B = BATCH // NCORES   # 64
T = DIM_INPUT         # 128 timesteps
KJ = 4
OJ = 2
NPH = 2               # psum phases

F32 = mybir.dt.float32
MMDT = mybir.dt.float16
MMNP = np.float16

# ---- schedule (sim.py search, mix-S-VV config; HW-validated best) ----
# Epilogue: scalar relu on window 0 [128,64]; vector relu on window 1
# [128,64] and on the window-2/3 pair tile [128,128].
EPI_CFG = [('S', [0]), ('V', [1]), ('V', [2, 3])]
# Per-step PE program order. HW rule: within one psum tile, accumulation
# groups are sequential (seed w .. stop w before seed w'); the pair tile
# (windows 3 then 2) obeys this; singles interleave freely. Trailing
# ('seed', w, 1) items are next-step seeds that fill the tail PE gap.
TEMPLATE = [('seed', 3, 0), ('mm', 3, 1), ('mm', 1, 1), ('mm', 0, 1),
            ('mm', 1, 0), ('mm', 0, 0), ('mm', 3, 0), ('mm', 0, 3),
            ('mm', 1, 2), ('mm', 1, 3), ('mm', 0, 2), ('mm', 3, 2),
            ('mm', 3, 3), ('seed', 2, 0), ('mm', 2, 2), ('mm', 2, 3),
            ('mm', 2, 0), ('mm', 2, 1), ('seed', 0, 1), ('seed', 1, 1)]


def _build_nc():
    nc = bacc.Bacc("TRN2", target_bir_lowering=False, debug=False,
                   num_devices=NCORES, num_swdge_queues=4)
    whAll = nc.dram_tensor("whAll", [128, KJ * DIM_REC], MMDT,
                           kind="ExternalInput")
    pack64 = nc.dram_tensor("pack64", [128, 64 + DIM_REC], MMDT,
                            kind="ExternalInput")
    pack128 = nc.dram_tensor("pack128", [128, 256], MMDT,
                             kind="ExternalInput")
    whyR = nc.dram_tensor("whyR", [128, KJ * DIM_OUT], MMDT,
                          kind="ExternalInput")
    byR = nc.dram_tensor("byR", [128, OJ], F32, kind="ExternalInput")
    yT = nc.dram_tensor("yT", [DIM_OUT, B], F32, kind="ExternalOutput")

    RELU = mybir.ActivationFunctionType.Relu
    IDENT = mybir.ActivationFunctionType.Identity
    ADD = mybir.AluOpType.add
    MAX = mybir.AluOpType.max

    # window -> (tile idx, col offset, tile width)
    wmap = {}
    for t, (_, wins) in enumerate(EPI_CFG):
        for i, w in enumerate(wins):
            wmap[w] = (t, i * 64, 64 * len(wins))

    with TileContext(nc) as tc:
        with tc.tile_pool(name="w", bufs=1) as wp, \
             tc.tile_pool(name="s", bufs=1) as sp, \
             tc.psum_pool(name="p", bufs=1) as pp:
            wh = [wp.tile([128, DIM_REC], MMDT, name=f"wh{k}")
                  for k in range(KJ)]
            eyet = wp.tile([128, 64], MMDT, name="eye")
            xhbt = wp.tile([128, DIM_REC], MMDT, name="xhbt")
            whyt = wp.tile([128, KJ * DIM_OUT], MMDT, name="why")
            byt = wp.tile([128, OJ], F32, name="byt")
            g0 = [wp.tile([128, 128], MMDT, name=f"g0i{pr}") for pr in range(2)]
            g = [[sp.tile([128, 128], MMDT, name=f"g{p}_{pr}")
                  for pr in range(2)] for p in range(2)]
            ps = [[pp.tile([128, 64 * len(wins)], F32, name=f"ps{q}_{t}")
                   for t, (_, wins) in enumerate(EPI_CFG)]
                  for q in range(NPH)]
            if any(it[0] == 'fill' for it in TEMPLATE):
                scratch = pp.tile([128, 64], F32, name="scratch")
            # tail projection reuses the phase tiles of windows 0/1 (their
            # last step-loop accumulation group is long closed by then)
            qy = (T + 1) % NPH
            psy = []
            for j in range(OJ):
                t, off, _ = wmap[j]
                psy.append(ps[qy][t][:, off:off + 64])

            nc.sync.dma_start(out=eyet[:], in_=pack64[:, 0:64])
            nc.sync.dma_start(out=xhbt[:], in_=pack64[:, 64:576])
            nc.scalar.dma_start(out=g0[0][:], in_=pack128[:, 0:128])
            nc.scalar.dma_start(out=g0[1][:], in_=pack128[:, 128:256])
            nc.scalar.dma_start(out=wh[1][:], in_=whAll[:, 512:1024])
            nc.sync.dma_start(out=wh[0][:], in_=whAll[:, 0:512])
            nc.gpsimd.dma_start(out=wh[3][:], in_=whAll[:, 1536:2048])
            nc.gpsimd.dma_start(out=wh[2][:], in_=whAll[:, 1024:1536])
            nc.gpsimd.dma_start(out=whyt[:], in_=whyR[:])
            nc.gpsimd.dma_start(out=byt[:], in_=byR[:])

            def gsrc(s, k):
                gt = g0[k // 2] if s == 1 else g[s % 2][k // 2]
                return gt[:, (k % 2) * 64:(k % 2) * 64 + 64]

            emitted_seed = set()

            def emit_seed(s_tgt, w):
                if s_tgt > T or (s_tgt, w) in emitted_seed:
                    return
                emitted_seed.add((s_tgt, w))
                t, off, _ = wmap[w]
                tile = ps[s_tgt % NPH][t]
                nc.tensor.matmul(tile[:, off:off + 64],
                                 xhbt[:, w * 128:(w + 1) * 128],
                                 eyet[:], start=True, stop=False)

            for s in range(2, T + 1):
                q = s % NPH
                cnt = [0] * 4
                for it in TEMPLATE:
                    if it[0] == 'seed':
                        emit_seed(s + it[2], it[1])
                    elif it[0] == 'fill':
                        for _ in range(it[1]):
                            nc.tensor.matmul(scratch[:],
                                             xhbt[:, 0:128],
                                             eyet[:],
                                             start=True, stop=True)
                    else:
                        _, j, k = it
                        emit_seed(s, j)
                        cnt[j] += 1
                        t, off, _ = wmap[j]
                        nc.tensor.matmul(
                            ps[q][t][:, off:off + 64],
                            wh[k][:, j * 128:(j + 1) * 128],
                            gsrc(s - 1, k),
                            start=False, stop=(cnt[j] == 4))
                for t, (eng, wins) in enumerate(EPI_CFG):
                    pr = wins[0] // 2
                    c0 = (wins[0] % 2) * 64
                    width = 64 * len(wins)
                    src = ps[q][t][:]
                    dst = g[s % 2][pr][:, c0:c0 + width]
                    if eng == 'S':
                        nc.scalar.activation(dst, src, RELU)
                    else:
                        nc.vector.tensor_scalar(dst, src, 0.0, None, MAX)

            for j in range(OJ):
                for k in range(KJ):
                    nc.tensor.matmul(
                        psy[j],
                        whyt[:, k * DIM_OUT + j * 128:k * DIM_OUT + (j + 1) * 128],
                        gsrc(T, k), start=(k == 0), stop=(k == KJ - 1))
            ytile = [sp.tile([128, B], F32, name=f"yt{j}") for j in range(OJ)]
            nc.scalar.activation(ytile[0][:], psy[0], IDENT,
                                 bias=byt[:, 0:1])
            nc.vector.tensor_scalar(ytile[1][:], psy[1], byt[:, 1:2],
                                    None, ADD)
            nc.sync.dma_start(out=yT[0:128, :], in_=ytile[0][:])
            nc.scalar.dma_start(out=yT[128:256, :], in_=ytile[1][:])

    nc.compile()
    return nc


_NC = None
TRACE = False
TRACE_TMPDIR = None
LAST_RESULTS = None


def kernel(x, W_x2h, b_x2h, W_h2h, b_h2h, W_h2y, b_h2y):
    global _NC, LAST_RESULTS
    if _NC is None:
        _NC = _build_nc()

    x = np.asarray(x, np.float32)
    Wx = np.asarray(W_x2h, np.float32)
    Wh = np.asarray(W_h2h, np.float32)
    WhyT = np.asarray(W_h2y, np.float32).T.astype(MMNP)
    bc = np.asarray(b_x2h, np.float32) + np.asarray(b_h2h, np.float32)

    WhT = Wh.T.astype(MMNP)
    shared = {
        "whAll": np.ascontiguousarray(np.concatenate(
            [WhT[k * 128:(k + 1) * 128, :] for k in range(KJ)], axis=1)),
        "whyR": np.ascontiguousarray(np.concatenate(
            [WhyT[k * 128:(k + 1) * 128, :] for k in range(KJ)], axis=1)),
        "byR": np.ascontiguousarray(
            np.asarray(b_h2y, np.float32).reshape(OJ, 128).T),
    }
    eye = np.eye(128, 64, dtype=MMNP)
    ins = []
    for i in range(NCORES):
        xc = x[i * B:(i + 1) * B, :]
        xhb = xc @ Wx.T + bc
        h1 = np.maximum(xhb, 0.0).T
        m = dict(shared)
        xhb_pad = np.zeros((128, DIM_REC), np.float32)
        xhb_pad[:64] = xhb
        m["pack64"] = np.ascontiguousarray(
            np.concatenate([eye, xhb_pad.astype(MMNP)], axis=1))
        m["pack128"] = np.ascontiguousarray(np.concatenate(
            [h1[0:128], h1[128:256], h1[256:384], h1[384:512]],
            axis=1).astype(MMNP))
        ins.append(m)

    kw = {}
    if TRACE:
        kw = {"trace": True, "tmpdir": TRACE_TMPDIR}
    res = run_bass_kernel_spmd(_NC, ins, core_ids=list(range(NCORES)), **kw)
    LAST_RESULTS = res
    out = np.empty((BATCH, DIM_OUT), np.float32)
    for i in range(NCORES):
        out[i * B:(i + 1) * B, :] = res.results[i]["yT"].T
    return out
